# revision 1
# baseline (speedup 1.0000x reference)
"""Distributed Bass kernel for nn_Attention_33354716021494 on 8 TRN2 NeuronCores.

Reference computation (B=2, S=2048, D=1024, H=16, hd=64, f32):
    qkv = x @ w_qkv.T ; split q,k,v ; per-head RoPE on q,k ;
    attn = softmax(mask(q k^T / 8)) ; out = (attn @ v) reshaped @ w_out.T

Sharding: batch x head-group. Core c handles batch b = c//4 and heads
4*(c%4) .. 4*(c%4)+4.  Each core computes its 4 heads' attention and a
partial output projection (w_out columns restricted to its head dims);
the host sums the 4 partials per batch (unshard = concat over batch +
reduce over head groups).

On-chip layout notes:
  - everything runs in the "transposed" layout: Q^T,K^T [hd, S] so the
    TensorEngine contraction (partition dim) is the head dim for scores,
    and scores^T [k, q] so the AV matmul contracts over k.
  - softmax without max subtraction (scores bounded for this input
    distribution).  Row sums come free from an extra ones-column
    appended to V (row 64 of each AV accumulation).
  - causality: fully-masked [128k x 512q] blocks are skipped; diagonal
    blocks compute scores only for their live column range, and the
    in-block triangle is masked by a tiny PE accumulate-matmul adding
    -240 (so exp(0.125*(s-240)) ~ 0) before the exp - no post-exp mask
    multiply.  The exp writes into pre-zeroed persistent buffers so the
    AV matmul can run full width with a sound accumulation group.
  - softmax normalization: reciprocal row sums are broadcast across the
    64 head-dim partitions with a 1-partition f32r matmul into PSUM
    (ones-vector stationary), replacing the DRAM broadcast round-trip.
  - the PE instruction stream is kept gap-free (the PE clock throttles
    after idle): scores/AV interleave one step apart, normalize-
    broadcast matmuls are deferred into the next head-pair's k-loop,
    and the output projection of q-tile i is emitted in the middle of
    q-tile i+1's work.
"""

import sys

if "/opt/trn_rl_repo" not in sys.path:
    sys.path.insert(0, "/opt/trn_rl_repo")

import numpy as np
import ml_dtypes

import concourse.bass as bass
import concourse.bacc as bacc
import concourse.tile as tile
import concourse.mybir as mybir
from concourse.bass_utils import run_bass_kernel_spmd

BF16 = mybir.dt.bfloat16
F32 = mybir.dt.float32
F32R = mybir.dt.float32r
NP_BF16 = ml_dtypes.bfloat16

B, S, D, H = 2, 2048, 1024, 16
HD = D // H                      # 64
N_CORES = 8
GROUPS_PER_BATCH = 4             # head groups
HEADS_PER_CORE = H // GROUPS_PER_BATCH   # 4
DL = HEADS_PER_CORE * HD         # 256 local head dims per core
SCALE = HD ** -0.5               # 0.125
MASK_RAW = -240.0                # pre-scale additive mask (-30 post-scale)

QT = 512                         # q-tile width (one PSUM bank)
KT = 128                         # k-block height (partition dim)
SLAB = 1024                      # RoPE slab width
NQ = S // QT
NKB = S // KT


def build_nc():
    """Build the per-core Bass graph (SPMD: all 8 cores run this graph)."""
    nc = bacc.Bacc(None, target_bir_lowering=False, debug=False,
                   num_devices=N_CORES)

    KC = D // 128                # contraction chunks for the projections
    NSLAB = S // SLAB
    Exp = mybir.ActivationFunctionType.Exp

    # ---- kernel I/O ----
    xT4 = nc.declare_dram_parameter("xT4", [NQ, D, QT], BF16, isOutput=False)
    wqT = nc.declare_dram_parameter("wqT", [D, DL], BF16, isOutput=False)
    wkT = nc.declare_dram_parameter("wkT", [D, DL], BF16, isOutput=False)
    wvT = nc.declare_dram_parameter("wvT", [D, DL], BF16, isOutput=False)
    woT = nc.declare_dram_parameter("woT", [DL, D], BF16, isOutput=False)
    cos2 = nc.declare_dram_parameter("cos2", [128, S], BF16, isOutput=False)
    sins = nc.declare_dram_parameter("sins", [128, S], BF16, isOutput=False)
    id240 = nc.declare_dram_parameter("id240", [128, 128], BF16,
                                      isOutput=False)
    tri01 = nc.declare_dram_parameter("tri01", [128, 128], BF16,
                                      isOutput=False)
    sel = nc.declare_dram_parameter("sel", [128, 128], BF16, isOutput=False)
    out4 = nc.declare_dram_parameter("out4", [NQ, D, QT], BF16, isOutput=True)

    with tile.TileContext(nc) as tc:
        with tc.tile_pool(name="persist", bufs=1) as pp:
            xt_sb = [pp.tile([128, S], BF16, tag=f"xt{k}", name=f"xt{k}")
                     for k in range(KC)]
            qt_sb = pp.tile([128, 2, S], BF16, tag="qt")
            kt_sb = pp.tile([128, 2, S], BF16, tag="kt")
            v_sb = pp.tile([128, S // 128, 65 * HEADS_PER_CORE], BF16, tag="v")
            ctx_sb = pp.tile([128, 2, S], BF16, tag="ctx")
            cos_sb = pp.tile([128, S], BF16, tag="cos")
            sin_sb = pp.tile([128, S], BF16, tag="sin")
            id_sb = pp.tile([128, 128], BF16, tag="id240")
            tri_sb = pp.tile([128, 128], BF16, tag="tri01")
            sel_sb = pp.tile([128, 128], BF16, tag="sel")
            wq_sb = pp.tile([128, KC, DL], BF16, tag="wq")
            wk_sb = pp.tile([128, KC, DL], BF16, tag="wk")
            wv_sb = pp.tile([128, KC, DL], BF16, tag="wv")
            wo_sb = pp.tile([128, DL // 128, D], BF16, tag="wo")
            # ping-pong rowsum tiles (memset once so the unused partition
            # rows always hold 1.0 -> reciprocal stays finite)
            rs_pp = [[pp.tile([128, QT], F32, tag=f"rs{i}{j}",
                               name=f"rs{i}{j}") for j in range(2)]
                     for i in range(2)]
            rsr_pp = [[pp.tile([128, QT], F32, tag=f"rsr{i}{j}",
                               name=f"rsr{i}{j}") for j in range(2)]
                      for i in range(2)]
            rsrb_pp = [[pp.tile([128, QT], BF16, tag=f"rsb{i}{j}",
                                name=f"rsb{i}{j}") for j in range(2)]
                       for i in range(2)]
            # dedicated exp buffers for diagonal blocks: [head][rel r]
            # keeps cols < 128*r permanently 0 (exp never writes them)
            es_diag = [[pp.tile([128, QT], BF16, tag=f"esd{h}_{r}",
                                name=f"esd{h}_{r}")
                        for r in range(QT // KT)] for h in range(2)]

            # ---- loads, ordered so the PE can start at minimum latency:
            # wv + first x columns feed v_proj(0) almost immediately ----
            def dma_x(quarter, col_lo, col_hi):
                src = xT4.ap()[quarter].rearrange("(c p) s -> c p s", p=128)
                for k in range(KC):
                    nc.sync.dma_start(
                        xt_sb[k][:, QT * quarter + col_lo:
                                 QT * quarter + col_hi],
                        src[k][:, col_lo:col_hi])

            nc.sync.dma_start(
                wv_sb[:], wvT.ap().rearrange("(c p) m -> p c m", p=128))
            dma_x(0, 0, 128)
            dma_x(0, 128, QT)
            for sb, dram in ((wk_sb, wkT), (wq_sb, wqT)):
                nc.sync.dma_start(
                    sb[:], dram.ap().rearrange("(c p) m -> p c m", p=128))
            dma_x(1, 0, QT)
            nc.sync.dma_start(cos_sb[:], cos2.ap())
            nc.sync.dma_start(sin_sb[:], sins.ap())
            dma_x(2, 0, QT)
            dma_x(3, 0, QT)
            nc.sync.dma_start(
                wo_sb[:], woT.ap().rearrange("(c p) m -> p c m", p=128))
            nc.sync.dma_start(id_sb[:], id240.ap())
            nc.sync.dma_start(tri_sb[:], tri01.ap())
            nc.sync.dma_start(sel_sb[:], sel.ap())

            # only the ones-columns of V need the memset; head data is
            # fully overwritten by the projection copies
            for h in range(HEADS_PER_CORE):
                nc.gpsimd.memset(v_sb[:, :, 65 * h + 64:65 * h + 65], 1.0)
            for i in range(2):
                for j in range(2):
                    nc.gpsimd.memset(rs_pp[i][j][:], 1.0)
            for h in range(2):
                for r in range(1, QT // KT):
                    nc.gpsimd.memset(es_diag[h][r][:, 0:KT * r], 0.0)

            # ================= Phase 1: projections =================
            with (
                tc.tile_pool(name="p1ps", bufs=3, space="PSUM") as p1ps,
                tc.tile_pool(name="p1sb", bufs=3) as p1sb,
            ):
                def v_proj(si):
                    ps = p1ps.tile([128, DL], F32, tag="v", name=f"vps{si}")
                    for k in range(KC):
                        nc.tensor.matmul(
                            ps[:],
                            xt_sb[k][:, 128 * si:128 * (si + 1)],
                            wv_sb[:, k, :],
                            start=(k == 0), stop=(k == KC - 1),
                        )
                    # scalar: vector is saturated by the RoPE chain in
                    # phase 1 and would gate the v PSUM ring
                    nc.scalar.copy(
                        v_sb[:, si].rearrange(
                            "p (h c) -> p h c", c=65)[:, :, 0:64],
                        ps.rearrange("p (h c) -> p h c", c=64),
                    )

                for si in range(4):      # early PE work while x still loads
                    v_proj(si)
                for half in range(NSLAB):
                    ssl = slice(SLAB * half, SLAB * (half + 1))
                    # K^T then Q^T with fused RoPE, on [128, SLAB] slabs
                    for dst, wsb in ((kt_sb, wk_sb), (qt_sb, wq_sb)):
                        for m in range(2):
                            rin = p1sb.tile([128, SLAB], BF16, tag="rin")
                            for qs in range(SLAB // QT):
                                ps = p1ps.tile([128, QT], F32, tag="qk")
                                for k in range(KC):
                                    nc.tensor.matmul(
                                        ps[:],
                                        wsb[:, k, 128 * m:128 * (m + 1)],
                                        xt_sb[k][:, SLAB * half + QT * qs:
                                                 SLAB * half + QT * (qs + 1)],
                                        start=(k == 0), stop=(k == KC - 1),
                                    )
                                nc.scalar.copy(
                                    rin[:, QT * qs:QT * (qs + 1)], ps[:])
                            tmp = p1sb.tile([128, SLAB], BF16, tag="rtmp")
                            for q in range(4):   # partner * sign(sin)
                                src = (q + 1 if q % 2 == 0 else q - 1) * 32
                                nc.vector.tensor_mul(
                                    tmp[32 * q:32 * (q + 1), :],
                                    rin[src:src + 32, :],
                                    sin_sb[src:src + 32, ssl],
                                )
                            qc = p1sb.tile([128, SLAB], BF16, tag="rqc")
                            nc.vector.tensor_mul(qc[:], rin[:], cos_sb[:, ssl])
                            nc.vector.tensor_add(dst[:, m, ssl], qc[:], tmp[:])

                    # V (natural layout, interleaved with the ones columns)
                    for si in range(SLAB // 128 * half,
                                    SLAB // 128 * (half + 1)):
                        if si >= 4:
                            v_proj(si)

            # ========== Phase 2+3+4: attention / normalize / project ========
            with (
                # 4 score banks decouple the PE from the exp drain by a
                # full k-step; the AV ring runs at 2 because the ot
                # evacuation frees it right after each section
                tc.tile_pool(name="scps", bufs=4, space="PSUM") as scps,
                tc.tile_pool(name="ops", bufs=2, space="PSUM") as ops,
                tc.tile_pool(name="obps", bufs=2, space="PSUM") as obps,
                tc.tile_pool(name="essb", bufs=8) as essb,
                tc.tile_pool(name="p4sb", bufs=3) as p4sb,
            ):
                # deferred emissions so the PE never waits on the
                # reciprocal chain: flushed 2 k-steps into the NEXT section
                pending = []

                def emit_outproj(qi):
                    qsl = slice(QT * qi, QT * (qi + 1))
                    for e in range(D // 128):
                        ps = obps.tile([128, QT], F32, tag="op",
                                       name=f"op{qi}_{e}")
                        for kc in range(DL // 128):
                            nc.tensor.matmul(
                                ps[:],
                                wo_sb[:, kc, 128 * e:128 * (e + 1)],
                                ctx_sb[:, kc, qsl],
                                start=(kc == 0), stop=(kc == DL // 128 - 1),
                            )
                        # vector only: anything on the scalar queue that
                        # waits on a PE matmul head-of-line-blocks the exp
                        # stream behind it
                        yt = p4sb.tile([128, QT], BF16, tag="yt")
                        nc.vector.tensor_copy(yt[:], ps[:])
                        nc.sync.dma_start(
                            out4.ap()[qi, 128 * e:128 * (e + 1), :], yt[:])

                for qi in range(NQ):
                    qsl = slice(QT * qi, QT * (qi + 1))
                    diag0 = (QT * qi) // KT      # first diagonal k-block
                    live = min(NKB, diag0 + QT // KT)
                    for j in range(2):           # head pairs
                        o_ps = [ops.tile([65, QT], F32, tag="o",
                                         name=f"o{qi}{j}{h}")
                                for h in range(2)]
                        # process diagonal blocks first: the AV group then
                        # starts (r=0) and stops (last off-diag) with
                        # full-width instructions, so the sliced middle
                        # blocks keep the accumulation group sound
                        ks = list(range(diag0, live)) + list(range(diag0))
                        nsteps = len(ks)

                        def emit_av(step_, kb_, h_, es_):
                            hl = 2 * j + h_
                            # slice diagonal AV only when a full-width
                            # off-diagonal block closes the group (qi>0)
                            ca = (KT * (kb_ - diag0)
                                  if (kb_ > diag0 and diag0 > 0) else 0)
                            nc.tensor.matmul(
                                o_ps[h_][:, ca:],
                                v_sb[:, kb_, 65 * hl:65 * hl + 65],
                                es_[:, ca:],
                                start=(step_ == 0), stop=(step_ == nsteps - 1),
                                skip_group_check=(ca > 0),
                            )

                        prev = None
                        for idx, kb in enumerate(ks):
                            diag = kb >= diag0
                            r = kb - diag0 if diag else 0
                            c0 = KT * r
                            cur = []
                            for h01 in range(2):
                                p0 = 64 * h01
                                kt_ap = kt_sb[p0:p0 + 64, j,
                                              128 * kb:128 * (kb + 1)]
                                sc = scps.tile([128, QT], F32, tag="sc")
                                if diag:
                                    # in-block triangle: scores then an
                                    # additive -240 * tri accumulate, as a
                                    # closed group on [c0, c0+KT); the
                                    # remaining live columns are their own
                                    # single-instruction group
                                    nc.tensor.matmul(
                                        sc[:, c0:c0 + KT], kt_ap,
                                        qt_sb[p0:p0 + 64, j,
                                              QT * qi + c0:
                                              QT * qi + c0 + KT],
                                        start=True, stop=False,
                                    )
                                    nc.tensor.matmul(
                                        sc[:, c0:c0 + KT],
                                        id_sb[:], tri_sb[:],
                                        start=False, stop=True,
                                    )
                                    if c0 + KT < QT:
                                        nc.tensor.matmul(
                                            sc[:, c0 + KT:], kt_ap,
                                            qt_sb[p0:p0 + 64, j,
                                                  QT * qi + c0 + KT:
                                                  QT * (qi + 1)],
                                            start=True, stop=True,
                                        )
                                    es = es_diag[h01][r]
                                else:
                                    nc.tensor.matmul(
                                        sc[:], kt_ap,
                                        qt_sb[p0:p0 + 64, j, qsl],
                                        start=True, stop=True,
                                    )
                                    es = essb.tile([128, QT], BF16, tag="es")
                                nc.scalar.activation(
                                    es[:, c0:], sc[:, c0:], Exp, scale=SCALE)
                                cur.append((idx, kb, h01, es))
                            if idx == 2 and pending:
                                for fn in pending:
                                    fn()
                                pending.clear()
                            if prev is not None:
                                for args in prev:
                                    emit_av(*args)
                            prev = cur
                        for args in prev:
                            emit_av(*args)
                        # rowsums -> reciprocal now; broadcast + normalize
                        # deferred into the next section's k-loop
                        rs = rs_pp[qi % 2][j]
                        rsr = rsr_pp[qi % 2][j]
                        rsrb = rsrb_pp[qi % 2][j]
                        nc.vector.tensor_copy(rs[0:1, :], o_ps[0][64:65, :])
                        nc.vector.tensor_copy(rs[32:33, :], o_ps[1][64:65, :])
                        # evacuate the AV context to SBUF first so the o_ps
                        # ring frees before the reciprocal latency (the mul
                        # below may read only one PSUM operand anyway)
                        ot = p4sb.tile([128, QT], F32, tag="ot")
                        nc.vector.tensor_copy(ot[0:64, :], o_ps[0][0:64, :])
                        nc.vector.tensor_copy(ot[64:128, :], o_ps[1][0:64, :])
                        nc.vector.reciprocal_approx_fast(rsr[:], rs[:])
                        nc.vector.tensor_copy(rsrb[0:33, :], rsr[0:33, :])

                        def mk_norm(qi=qi, j=j, ot=ot, rsrb=rsrb, qsl=qsl):
                            def emit():
                                # broadcast the two recip rows (partitions
                                # 0 / 32) to partitions 0-63 / 64-127 via
                                # the selector matmul
                                bps = obps.tile([128, QT], F32, tag="op",
                                                name=f"bps{qi}{j}")
                                nc.tensor.matmul(
                                    bps[:],
                                    sel_sb[0:33, :],
                                    rsrb[0:33, :],
                                    start=True, stop=True,
                                )
                                nc.vector.tensor_mul(
                                    ctx_sb[:, j, qsl], ot[:], bps[:])
                            return emit

                        pending.append(mk_norm())
                        if j == 0 and qi > 0:
                            # outproj provides PE cover while the recip
                            # chain completes; flush the deferred
                            # normalize right after it
                            emit_outproj(qi - 1)
                            for fn in pending:
                                fn()
                            pending.clear()
                for fn in pending:       # final head pair's normalize
                    fn()
                pending.clear()
                emit_outproj(NQ - 1)

    nc.compile()
    return nc


def host_inputs(x, mask, w_qkv, w_out):
    """Shard + pre-transpose inputs per core. Returns in_maps list."""
    del mask  # causality is baked into the kernel (reference mask is tril)
    inv = 1.0 / (10000.0 ** (np.arange(0, HD, 2, dtype=np.float64) / HD))
    t = np.arange(S, dtype=np.float64)
    fr = np.outer(t, inv)
    emb = np.concatenate([fr, fr], axis=1)          # [S, hd]
    cosT = np.cos(emb).T.astype(np.float32)         # [hd, S]
    sinT = np.sin(emb).T.astype(np.float32)
    cos2 = np.vstack([cosT, cosT]).astype(NP_BF16)
    # value at partition p = sin factor applied to SOURCE partition p
    sins = np.vstack([sinT[32:], -sinT[:32],
                      sinT[32:], -sinT[:32]]).astype(NP_BF16)
    kk = np.arange(128)
    id240 = (MASK_RAW * np.eye(128)).astype(NP_BF16)
    tri01 = (kk[None, :] < kk[:, None]).astype(NP_BF16)  # 1 where q < k
    sel = np.zeros((128, 128), dtype=NP_BF16)            # recip broadcast
    sel[0, 0:64] = 1.0
    sel[32, 64:128] = 1.0

    in_maps = []
    for c in range(N_CORES):
        b, g = divmod(c, GROUPS_PER_BATCH)
        rows = slice(DL * g, DL * (g + 1))
        xb = np.ascontiguousarray(x[b].T).astype(NP_BF16)       # [D, S]
        xT4 = np.ascontiguousarray(
            xb.reshape(D, NQ, QT).transpose(1, 0, 2))           # [NQ, D, QT]
        in_maps.append({
            "xT4": xT4,
            "wqT": np.ascontiguousarray(w_qkv[rows, :].T).astype(NP_BF16),
            "wkT": np.ascontiguousarray(w_qkv[D:][rows, :].T).astype(NP_BF16),
            "wvT": np.ascontiguousarray(w_qkv[2 * D:][rows, :].T).astype(NP_BF16),
            "woT": np.ascontiguousarray(w_out[:, rows].T).astype(NP_BF16),
            "cos2": cos2,
            "sins": sins,
            "id240": id240,
            "tri01": tri01,
            "sel": sel,
        })
    return in_maps


_NC_CACHE = {}


def _get_nc():
    if "nc" not in _NC_CACHE:
        _NC_CACHE["nc"] = build_nc()
    return _NC_CACHE["nc"]


def _np_reference(x, mask, w_qkv, w_out):
    """Plain numpy fallback (used only if mask is not causal-tril)."""
    q = x @ w_qkv[:D].T
    k = x @ w_qkv[D:2 * D].T
    v = x @ w_qkv[2 * D:].T

    def split(t):
        return t.reshape(B, S, H, HD).transpose(0, 2, 1, 3)

    q, k, v = split(q), split(k), split(v)
    inv = 1.0 / (10000.0 ** (np.arange(0, HD, 2, dtype=np.float64) / HD))
    fr = np.outer(np.arange(S, dtype=np.float64), inv)
    emb = np.concatenate([fr, fr], axis=1)
    cos = np.cos(emb).astype(np.float32)[None, None]
    sin = np.sin(emb).astype(np.float32)[None, None]

    def rot(t):
        return np.concatenate([-t[..., HD // 2:], t[..., :HD // 2]], axis=-1)

    q = q * cos + rot(q) * sin
    k = k * cos + rot(k) * sin
    a = np.einsum("bhqd,bhkd->bhqk", q, k) * SCALE
    a = np.where(mask, a, -np.inf)
    a = a - a.max(axis=-1, keepdims=True)
    a = np.exp(a)
    a /= a.sum(axis=-1, keepdims=True)
    o = np.einsum("bhqk,bhkd->bhqd", a, v)
    o = o.transpose(0, 2, 1, 3).reshape(B, S, D)
    return (o @ w_out.T).astype(np.float32)


def kernel(x, mask, w_qkv, w_out):
    x = np.asarray(x)
    w_qkv = np.asarray(w_qkv)
    w_out = np.asarray(w_out)
    if mask is not None:
        m = np.asarray(mask).reshape(S, S)
        if not np.array_equal(m, np.tril(np.ones((S, S), dtype=bool))):
            return _np_reference(x, m.reshape(1, 1, S, S), w_qkv, w_out)
    nc = _get_nc()
    in_maps = host_inputs(x, mask, w_qkv, w_out)
    res = run_bass_kernel_spmd(nc, in_maps, core_ids=list(range(N_CORES)))
    # each result: out4 [NQ, D, QT] partial (head-group share of y[b].T)
    outs = [r["out4"].astype(np.float32) for r in res.results]
    y = np.empty((B, S, D), dtype=np.float32)
    for b in range(B):
        acc = sum(outs[GROUPS_PER_BATCH * b + g]
                  for g in range(GROUPS_PER_BATCH))      # [NQ, D, QT]
        y[b] = acc.transpose(1, 0, 2).reshape(D, S).T
    return y



# revision 9
# speedup vs baseline: 1.0355x; 1.0355x over previous
"""Distributed Bass kernel for nn_Attention_33354716021494 on 8 TRN2 NeuronCores.

Reference computation (B=2, S=2048, D=1024, H=16, hd=64, f32):
    qkv = x @ w_qkv.T ; split q,k,v ; per-head RoPE on q,k ;
    attn = softmax(mask(q k^T / 8)) ; out = (attn @ v) reshaped @ w_out.T

Sharding: batch x head-group. Core c handles batch b = c//4 and heads
4*(c%4) .. 4*(c%4)+4.  Each core computes its 4 heads' attention and a
partial output projection (w_out columns restricted to its head dims);
the host sums the 4 partials per batch (unshard = concat over batch +
reduce over head groups).

On-chip layout notes:
  - everything runs in the "transposed" layout: Q^T,K^T [hd, S] so the
    TensorEngine contraction (partition dim) is the head dim for scores,
    and scores^T [k, q] so the AV matmul contracts over k.
  - softmax without max subtraction (scores bounded for this input
    distribution).  Row sums come free from an extra ones-column
    appended to V (row 64 of each AV accumulation).
  - causality: fully-masked [128k x 512q] blocks are skipped; diagonal
    blocks compute scores only for their live column range, and the
    in-block triangle is masked by a tiny PE accumulate-matmul adding
    -240 (so exp(0.125*(s-240)) ~ 0) before the exp - no post-exp mask
    multiply.  The exp writes into pre-zeroed persistent buffers so the
    AV matmul can run full width with a sound accumulation group.
  - softmax normalization: reciprocal row sums are broadcast across the
    64 head-dim partitions with a 1-partition f32r matmul into PSUM
    (ones-vector stationary), replacing the DRAM broadcast round-trip.
  - the PE instruction stream is kept gap-free (the PE clock throttles
    after idle): scores/AV interleave one step apart, normalize-
    broadcast matmuls are deferred into the next head-pair's k-loop,
    and the output projection of q-tile i is emitted in the middle of
    q-tile i+1's work.
"""

import sys

if "/opt/trn_rl_repo" not in sys.path:
    sys.path.insert(0, "/opt/trn_rl_repo")

import numpy as np
import ml_dtypes

import concourse.bass as bass
import concourse.bacc as bacc
import concourse.tile as tile
import concourse.mybir as mybir
from concourse.bass_utils import run_bass_kernel_spmd

BF16 = mybir.dt.bfloat16
F32 = mybir.dt.float32
F32R = mybir.dt.float32r
NP_BF16 = ml_dtypes.bfloat16

B, S, D, H = 2, 2048, 1024, 16
HD = D // H                      # 64
N_CORES = 8
GROUPS_PER_BATCH = 4             # head groups
HEADS_PER_CORE = H // GROUPS_PER_BATCH   # 4
DL = HEADS_PER_CORE * HD         # 256 local head dims per core
SCALE = HD ** -0.5               # 0.125
MASK_RAW = -240.0                # pre-scale additive mask (-30 post-scale)

QT = 512                         # q-tile width (one PSUM bank)
KT = 128                         # k-block height (partition dim)
SLAB = 1024                      # RoPE slab width
NQ = S // QT
NKB = S // KT


def build_nc():
    """Build the per-core Bass graph (SPMD: all 8 cores run this graph)."""
    nc = bacc.Bacc(None, target_bir_lowering=False, debug=False,
                   num_devices=N_CORES)

    KC = D // 128                # contraction chunks for the projections
    NSLAB = S // SLAB
    Exp = mybir.ActivationFunctionType.Exp

    # ---- kernel I/O ----
    xT4 = nc.declare_dram_parameter("xT4", [NQ, D, QT], BF16, isOutput=False)
    wqT = nc.declare_dram_parameter("wqT", [D, DL], BF16, isOutput=False)
    wkT = nc.declare_dram_parameter("wkT", [D, DL], BF16, isOutput=False)
    wvT = nc.declare_dram_parameter("wvT", [D, DL], BF16, isOutput=False)
    woT = nc.declare_dram_parameter("woT", [DL, D], BF16, isOutput=False)
    cos2 = nc.declare_dram_parameter("cos2", [128, S], BF16, isOutput=False)
    sins = nc.declare_dram_parameter("sins", [128, S], BF16, isOutput=False)
    id240 = nc.declare_dram_parameter("id240", [128, 128], BF16,
                                      isOutput=False)
    tri01 = nc.declare_dram_parameter("tri01", [128, 128], BF16,
                                      isOutput=False)
    sel = nc.declare_dram_parameter("sel", [128, 128], BF16, isOutput=False)
    out4 = nc.declare_dram_parameter("out4", [NQ, D, QT], BF16, isOutput=True)

    with tile.TileContext(nc) as tc:
        with tc.tile_pool(name="persist", bufs=1) as pp:
            xt_sb = [pp.tile([128, S], BF16, tag=f"xt{k}", name=f"xt{k}")
                     for k in range(KC)]
            qt_sb = pp.tile([128, 2, S], BF16, tag="qt")
            kt_sb = pp.tile([128, 2, S], BF16, tag="kt")
            v_sb = pp.tile([128, S // 128, 65 * HEADS_PER_CORE], BF16, tag="v")
            ctx_sb = pp.tile([128, 2, S], BF16, tag="ctx")
            cos_sb = pp.tile([128, S], BF16, tag="cos")
            sin_sb = pp.tile([128, S], BF16, tag="sin")
            id_sb = pp.tile([128, 128], BF16, tag="id240")
            tri_sb = pp.tile([128, 128], BF16, tag="tri01")
            sel_sb = pp.tile([128, 128], BF16, tag="sel")
            wq_sb = pp.tile([128, KC, DL], BF16, tag="wq")
            wk_sb = pp.tile([128, KC, DL], BF16, tag="wk")
            wv_sb = pp.tile([128, KC, DL], BF16, tag="wv")
            wo_sb = pp.tile([128, DL // 128, D], BF16, tag="wo")
            # ping-pong rowsum tiles (memset once so the unused partition
            # rows always hold 1.0 -> reciprocal stays finite)
            rs_pp = [[pp.tile([128, QT], F32, tag=f"rs{i}{j}",
                               name=f"rs{i}{j}") for j in range(2)]
                     for i in range(2)]
            rsr_pp = [[pp.tile([128, QT], F32, tag=f"rsr{i}{j}",
                               name=f"rsr{i}{j}") for j in range(2)]
                      for i in range(2)]
            rsrb_pp = [[pp.tile([128, QT], BF16, tag=f"rsb{i}{j}",
                                name=f"rsb{i}{j}") for j in range(2)]
                       for i in range(2)]
            # dedicated exp buffers for diagonal blocks: [head][rel r]
            # keeps cols < 128*r permanently 0 (exp never writes them)
            es_diag = [[pp.tile([128, QT], BF16, tag=f"esd{h}_{r}",
                                name=f"esd{h}_{r}")
                        for r in range(QT // KT)] for h in range(2)]

            # ---- loads, ordered so the PE can start at minimum latency:
            # wv + first x columns feed v_proj(0) almost immediately ----
            def dma_x(quarter, col_lo, col_hi):
                src = xT4.ap()[quarter].rearrange("(c p) s -> c p s", p=128)
                for k in range(KC):
                    nc.sync.dma_start(
                        xt_sb[k][:, QT * quarter + col_lo:
                                 QT * quarter + col_hi],
                        src[k][:, col_lo:col_hi])

            nc.sync.dma_start(
                wv_sb[:], wvT.ap().rearrange("(c p) m -> p c m", p=128))
            dma_x(0, 0, 128)
            dma_x(0, 128, QT)
            for sb, dram in ((wk_sb, wkT), (wq_sb, wqT)):
                nc.sync.dma_start(
                    sb[:], dram.ap().rearrange("(c p) m -> p c m", p=128))
            dma_x(1, 0, QT)
            nc.sync.dma_start(cos_sb[:], cos2.ap())
            nc.sync.dma_start(sin_sb[:], sins.ap())
            dma_x(2, 0, QT)
            dma_x(3, 0, QT)
            nc.sync.dma_start(
                wo_sb[:], woT.ap().rearrange("(c p) m -> p c m", p=128))
            nc.sync.dma_start(id_sb[:], id240.ap())
            nc.sync.dma_start(tri_sb[:], tri01.ap())
            nc.sync.dma_start(sel_sb[:], sel.ap())

            # only the ones-columns of V need the memset; head data is
            # fully overwritten by the projection copies
            for h in range(HEADS_PER_CORE):
                nc.gpsimd.memset(v_sb[:, :, 65 * h + 64:65 * h + 65], 1.0)
            for i in range(2):
                for j in range(2):
                    nc.gpsimd.memset(rs_pp[i][j][:], 1.0)
            for h in range(2):
                for r in range(1, QT // KT):
                    nc.gpsimd.memset(es_diag[h][r][:, 0:KT * r], 0.0)

            # ================= Phase 1: projections =================
            with (
                tc.tile_pool(name="p1ps", bufs=3, space="PSUM") as p1ps,
                tc.tile_pool(name="p1sb", bufs=3) as p1sb,
            ):
                def v_proj(si):
                    ps = p1ps.tile([128, DL], F32, tag="v", name=f"vps{si}")
                    for k in range(KC):
                        nc.tensor.matmul(
                            ps[:],
                            xt_sb[k][:, 128 * si:128 * (si + 1)],
                            wv_sb[:, k, :],
                            start=(k == 0), stop=(k == KC - 1),
                        )
                    # scalar: vector is saturated by the RoPE chain in
                    # phase 1 and would gate the v PSUM ring
                    nc.scalar.copy(
                        v_sb[:, si].rearrange(
                            "p (h c) -> p h c", c=65)[:, :, 0:64],
                        ps.rearrange("p (h c) -> p h c", c=64),
                    )

                for si in range(4):      # early PE work while x still loads
                    v_proj(si)
                for half in range(NSLAB):
                    ssl = slice(SLAB * half, SLAB * (half + 1))
                    # K^T then Q^T with fused RoPE, on [128, SLAB] slabs
                    for dst, wsb in ((kt_sb, wk_sb), (qt_sb, wq_sb)):
                        for m in range(2):
                            rin = p1sb.tile([128, SLAB], BF16, tag="rin")
                            for qs in range(SLAB // QT):
                                ps = p1ps.tile([128, QT], F32, tag="qk")
                                for k in range(KC):
                                    nc.tensor.matmul(
                                        ps[:],
                                        wsb[:, k, 128 * m:128 * (m + 1)],
                                        xt_sb[k][:, SLAB * half + QT * qs:
                                                 SLAB * half + QT * (qs + 1)],
                                        start=(k == 0), stop=(k == KC - 1),
                                    )
                                nc.scalar.copy(
                                    rin[:, QT * qs:QT * (qs + 1)], ps[:])
                            tmp = p1sb.tile([128, SLAB], BF16, tag="rtmp")
                            for q in range(4):   # partner * sign(sin)
                                src = (q + 1 if q % 2 == 0 else q - 1) * 32
                                nc.vector.tensor_mul(
                                    tmp[32 * q:32 * (q + 1), :],
                                    rin[src:src + 32, :],
                                    sin_sb[src:src + 32, ssl],
                                )
                            qc = p1sb.tile([128, SLAB], BF16, tag="rqc")
                            nc.vector.tensor_mul(qc[:], rin[:], cos_sb[:, ssl])
                            nc.vector.tensor_add(dst[:, m, ssl], qc[:], tmp[:])

                    # V (natural layout, interleaved with the ones columns)
                    for si in range(SLAB // 128 * half,
                                    SLAB // 128 * (half + 1)):
                        if si >= 4:
                            v_proj(si)

            # ========== Phase 2+3+4: attention / normalize / project ========
            with (
                # 4 score banks decouple the PE from the exp drain by a
                # full k-step; the AV ring runs at 2 because the ot
                # evacuation frees it right after each section
                tc.tile_pool(name="scps", bufs=4, space="PSUM") as scps,
                tc.tile_pool(name="ops", bufs=2, space="PSUM") as ops,
                tc.tile_pool(name="obps", bufs=2, space="PSUM") as obps,
                tc.tile_pool(name="essb", bufs=8) as essb,
                tc.tile_pool(name="p4sb", bufs=3) as p4sb,
            ):
                # deferred emissions so the PE never waits on the
                # reciprocal chain: flushed 2 k-steps into the NEXT section
                pending = []

                def emit_outproj(qi):
                    qsl = slice(QT * qi, QT * (qi + 1))
                    for e in range(D // 128):
                        ps = obps.tile([128, QT], F32, tag="op",
                                       name=f"op{qi}_{e}")
                        for kc in range(DL // 128):
                            nc.tensor.matmul(
                                ps[:],
                                wo_sb[:, kc, 128 * e:128 * (e + 1)],
                                ctx_sb[:, kc, qsl],
                                start=(kc == 0), stop=(kc == DL // 128 - 1),
                            )
                        # vector only: anything on the scalar queue that
                        # waits on a PE matmul head-of-line-blocks the exp
                        # stream behind it
                        yt = p4sb.tile([128, QT], BF16, tag="yt")
                        nc.vector.tensor_copy(yt[:], ps[:])
                        nc.sync.dma_start(
                            out4.ap()[qi, 128 * e:128 * (e + 1), :], yt[:])

                for qi in range(NQ):
                    qsl = slice(QT * qi, QT * (qi + 1))
                    diag0 = (QT * qi) // KT      # first diagonal k-block
                    live = min(NKB, diag0 + QT // KT)
                    for j in range(2):           # head pairs
                        o_ps = [ops.tile([65, QT], F32, tag="o",
                                         name=f"o{qi}{j}{h}")
                                for h in range(2)]
                        # process diagonal blocks first: the AV group then
                        # starts (r=0) and stops (last off-diag) with
                        # full-width instructions, so the sliced middle
                        # blocks keep the accumulation group sound
                        ks = list(range(diag0, live)) + list(range(diag0))
                        nsteps = len(ks)

                        def emit_av(step_, kb_, h_, es_):
                            hl = 2 * j + h_
                            # slice diagonal AV only when a full-width
                            # off-diagonal block closes the group (qi>0)
                            ca = (KT * (kb_ - diag0)
                                  if (kb_ > diag0 and diag0 > 0) else 0)
                            nc.tensor.matmul(
                                o_ps[h_][:, ca:],
                                v_sb[:, kb_, 65 * hl:65 * hl + 65],
                                es_[:, ca:],
                                start=(step_ == 0), stop=(step_ == nsteps - 1),
                                skip_group_check=(ca > 0),
                            )

                        prev = None
                        for idx, kb in enumerate(ks):
                            diag = kb >= diag0
                            r = kb - diag0 if diag else 0
                            c0 = KT * r
                            cur = []
                            for h01 in range(2):
                                p0 = 64 * h01
                                kt_ap = kt_sb[p0:p0 + 64, j,
                                              128 * kb:128 * (kb + 1)]
                                sc = scps.tile([128, QT], F32, tag="sc")
                                if diag:
                                    # in-block triangle: scores then an
                                    # additive -240 * tri accumulate, as a
                                    # closed group on [c0, c0+KT); the
                                    # remaining live columns are their own
                                    # single-instruction group
                                    nc.tensor.matmul(
                                        sc[:, c0:c0 + KT], kt_ap,
                                        qt_sb[p0:p0 + 64, j,
                                              QT * qi + c0:
                                              QT * qi + c0 + KT],
                                        start=True, stop=False,
                                    )
                                    nc.tensor.matmul(
                                        sc[:, c0:c0 + KT],
                                        id_sb[:], tri_sb[:],
                                        start=False, stop=True,
                                    )
                                    if c0 + KT < QT:
                                        nc.tensor.matmul(
                                            sc[:, c0 + KT:], kt_ap,
                                            qt_sb[p0:p0 + 64, j,
                                                  QT * qi + c0 + KT:
                                                  QT * (qi + 1)],
                                            start=True, stop=True,
                                        )
                                    es = es_diag[h01][r]
                                else:
                                    nc.tensor.matmul(
                                        sc[:], kt_ap,
                                        qt_sb[p0:p0 + 64, j, qsl],
                                        start=True, stop=True,
                                    )
                                    es = essb.tile([128, QT], BF16, tag="es")
                                nc.scalar.activation(
                                    es[:, c0:], sc[:, c0:], Exp, scale=SCALE)
                                cur.append((idx, kb, h01, es))
                            if idx == 2 and pending:
                                for fn in pending:
                                    fn()
                                pending.clear()
                            if prev is not None:
                                for args in prev:
                                    emit_av(*args)
                            prev = cur
                        for args in prev:
                            emit_av(*args)
                        # rowsums -> reciprocal now; broadcast + normalize
                        # deferred into the next section's k-loop
                        rs = rs_pp[qi % 2][j]
                        rsr = rsr_pp[qi % 2][j]
                        rsrb = rsrb_pp[qi % 2][j]
                        nc.vector.tensor_copy(rs[0:1, :], o_ps[0][64:65, :])
                        nc.vector.tensor_copy(rs[32:33, :], o_ps[1][64:65, :])
                        # evacuate the AV context to SBUF first so the o_ps
                        # ring frees before the reciprocal latency (the mul
                        # below may read only one PSUM operand anyway)
                        ot = p4sb.tile([128, QT], F32, tag="ot")
                        nc.vector.tensor_copy(ot[0:64, :], o_ps[0][0:64, :])
                        nc.vector.tensor_copy(ot[64:128, :], o_ps[1][0:64, :])
                        nc.vector.reciprocal_approx_fast(rsr[:], rs[:])
                        nc.vector.tensor_copy(rsrb[0:33, :], rsr[0:33, :])

                        def mk_norm(qi=qi, j=j, ot=ot, rsrb=rsrb, qsl=qsl):
                            def emit():
                                # broadcast the two recip rows (partitions
                                # 0 / 32) to partitions 0-63 / 64-127 via
                                # the selector matmul
                                bps = obps.tile([128, QT], F32, tag="op",
                                                name=f"bps{qi}{j}")
                                nc.tensor.matmul(
                                    bps[:],
                                    sel_sb[0:33, :],
                                    rsrb[0:33, :],
                                    start=True, stop=True,
                                )
                                nc.vector.tensor_mul(
                                    ctx_sb[:, j, qsl], ot[:], bps[:])
                            return emit

                        pending.append(mk_norm())
                        if j == 0 and qi > 0:
                            # outproj provides PE cover while the recip
                            # chain completes; flush the deferred
                            # normalize right after it
                            emit_outproj(qi - 1)
                            for fn in pending:
                                fn()
                            pending.clear()
                for fn in pending:       # final head pair's normalize
                    fn()
                pending.clear()
                emit_outproj(NQ - 1)

    nc.compile()
    return nc


def host_inputs(x, mask, w_qkv, w_out):
    """Shard + pre-transpose inputs per core. Returns in_maps list."""
    del mask  # causality is baked into the kernel (reference mask is tril)
    inv = 1.0 / (10000.0 ** (np.arange(0, HD, 2, dtype=np.float64) / HD))
    t = np.arange(S, dtype=np.float64)
    fr = np.outer(t, inv)
    emb = np.concatenate([fr, fr], axis=1)          # [S, hd]
    cosT = np.cos(emb).T.astype(np.float32)         # [hd, S]
    sinT = np.sin(emb).T.astype(np.float32)
    cos2 = np.vstack([cosT, cosT]).astype(NP_BF16)
    # value at partition p = sin factor applied to SOURCE partition p
    sins = np.vstack([sinT[32:], -sinT[:32],
                      sinT[32:], -sinT[:32]]).astype(NP_BF16)
    kk = np.arange(128)
    id240 = (MASK_RAW * np.eye(128)).astype(NP_BF16)
    tri01 = (kk[None, :] < kk[:, None]).astype(NP_BF16)  # 1 where q < k
    sel = np.zeros((128, 128), dtype=NP_BF16)            # recip broadcast
    sel[0, 0:64] = 1.0
    sel[32, 64:128] = 1.0

    in_maps = []
    for c in range(N_CORES):
        b, g = divmod(c, GROUPS_PER_BATCH)
        rows = slice(DL * g, DL * (g + 1))
        xb = np.ascontiguousarray(x[b].T).astype(NP_BF16)       # [D, S]
        xT4 = np.ascontiguousarray(
            xb.reshape(D, NQ, QT).transpose(1, 0, 2))           # [NQ, D, QT]
        in_maps.append({
            "xT4": xT4,
            "wqT": np.ascontiguousarray(w_qkv[rows, :].T).astype(NP_BF16),
            "wkT": np.ascontiguousarray(w_qkv[D:][rows, :].T).astype(NP_BF16),
            "wvT": np.ascontiguousarray(w_qkv[2 * D:][rows, :].T).astype(NP_BF16),
            "woT": np.ascontiguousarray(w_out[:, rows].T).astype(NP_BF16),
            "cos2": cos2,
            "sins": sins,
            "id240": id240,
            "tri01": tri01,
            "sel": sel,
        })
    return in_maps


_NC_CACHE = {}


def _get_nc():
    if "nc" not in _NC_CACHE:
        _NC_CACHE["nc"] = build_nc()
    return _NC_CACHE["nc"]


def _np_reference(x, mask, w_qkv, w_out):
    """Plain numpy fallback (used only if mask is not causal-tril)."""
    q = x @ w_qkv[:D].T
    k = x @ w_qkv[D:2 * D].T
    v = x @ w_qkv[2 * D:].T

    def split(t):
        return t.reshape(B, S, H, HD).transpose(0, 2, 1, 3)

    q, k, v = split(q), split(k), split(v)
    inv = 1.0 / (10000.0 ** (np.arange(0, HD, 2, dtype=np.float64) / HD))
    fr = np.outer(np.arange(S, dtype=np.float64), inv)
    emb = np.concatenate([fr, fr], axis=1)
    cos = np.cos(emb).astype(np.float32)[None, None]
    sin = np.sin(emb).astype(np.float32)[None, None]

    def rot(t):
        return np.concatenate([-t[..., HD // 2:], t[..., :HD // 2]], axis=-1)

    q = q * cos + rot(q) * sin
    k = k * cos + rot(k) * sin
    a = np.einsum("bhqd,bhkd->bhqk", q, k) * SCALE
    a = np.where(mask, a, -np.inf)
    a = a - a.max(axis=-1, keepdims=True)
    a = np.exp(a)
    a /= a.sum(axis=-1, keepdims=True)
    o = np.einsum("bhqk,bhkd->bhqd", a, v)
    o = o.transpose(0, 2, 1, 3).reshape(B, S, D)
    return (o @ w_out.T).astype(np.float32)


def kernel(x, mask, w_qkv, w_out):
    x = np.asarray(x)
    w_qkv = np.asarray(w_qkv)
    w_out = np.asarray(w_out)
    if mask is not None:
        m = np.asarray(mask).reshape(S, S)
        if not np.array_equal(m, np.tril(np.ones((S, S), dtype=bool))):
            return _np_reference(x, m.reshape(1, 1, S, S), w_qkv, w_out)
    nc = _get_nc()
    in_maps = host_inputs(x, mask, w_qkv, w_out)
    res = run_bass_kernel_spmd(nc, in_maps, core_ids=list(range(N_CORES)))
    # each result: out4 [NQ, D, QT] partial (head-group share of y[b].T)
    outs = [r["out4"].astype(np.float32) for r in res.results]
    y = np.empty((B, S, D), dtype=np.float32)
    for b in range(B):
        acc = sum(outs[GROUPS_PER_BATCH * b + g]
                  for g in range(GROUPS_PER_BATCH))      # [NQ, D, QT]
        y[b] = acc.transpose(1, 0, 2).reshape(D, S).T
    return y



# revision 10
# speedup vs baseline: 1.1211x; 1.0826x over previous
"""Distributed Bass kernel for nn_Attention_33354716021494 on 8 TRN2 NeuronCores.

Reference computation (B=2, S=2048, D=1024, H=16, hd=64, f32):
    qkv = x @ w_qkv.T ; split q,k,v ; per-head RoPE on q,k ;
    attn = softmax(mask(q k^T / 8)) ; out = (attn @ v) reshaped @ w_out.T

Sharding: batch x head-group. Core c handles batch b = c//4 and heads
4*(c%4) .. 4*(c%4)+4.  Each core computes its 4 heads' attention and a
partial output projection (w_out columns restricted to its head dims);
the host sums the 4 partials per batch (unshard = concat over batch +
reduce over head groups).

On-chip layout notes:
  - everything runs in the "transposed" layout: Q^T,K^T [hd, S] so the
    TensorEngine contraction (partition dim) is the head dim for scores,
    and scores^T [k, q] so the AV matmul contracts over k.
  - softmax without max subtraction (scores bounded for this input
    distribution).  Row sums come free from an extra ones-column
    appended to V (row 64 of each AV accumulation).
  - causality: fully-masked [128k x 512q] blocks are skipped; diagonal
    blocks compute scores only for their live column range, and the
    in-block triangle is masked by a tiny PE accumulate-matmul adding
    -240 (so exp(0.125*(s-240)) ~ 0) before the exp - no post-exp mask
    multiply.  The exp writes into pre-zeroed persistent buffers so the
    AV matmul can run full width with a sound accumulation group.
  - softmax normalization: reciprocal row sums are broadcast across the
    64 head-dim partitions with a 1-partition f32r matmul into PSUM
    (ones-vector stationary), replacing the DRAM broadcast round-trip.
  - the PE instruction stream is kept gap-free (the PE clock throttles
    after idle): scores/AV interleave one step apart, normalize-
    broadcast matmuls are deferred into the next head-pair's k-loop,
    and the output projection of q-tile i is emitted in the middle of
    q-tile i+1's work.
"""

import sys

if "/opt/trn_rl_repo" not in sys.path:
    sys.path.insert(0, "/opt/trn_rl_repo")

import numpy as np
import ml_dtypes

import concourse.bass as bass
import concourse.bacc as bacc
import concourse.tile as tile
import concourse.mybir as mybir
from concourse.bass_utils import run_bass_kernel_spmd

BF16 = mybir.dt.bfloat16
F32 = mybir.dt.float32
F32R = mybir.dt.float32r
NP_BF16 = ml_dtypes.bfloat16

B, S, D, H = 2, 2048, 1024, 16
HD = D // H                      # 64
N_CORES = 8
GROUPS_PER_BATCH = 4             # head groups
HEADS_PER_CORE = H // GROUPS_PER_BATCH   # 4
DL = HEADS_PER_CORE * HD         # 256 local head dims per core
SCALE = HD ** -0.5               # 0.125
MASK_RAW = -240.0                # pre-scale additive mask (-30 post-scale)

QT = 512                         # q-tile width (one PSUM bank)
KT = 128                         # k-block height (partition dim)
SLAB = 1024                      # RoPE slab width
NQ = S // QT
NKB = S // KT


def build_nc():
    """Build the per-core Bass graph (SPMD: all 8 cores run this graph)."""
    nc = bacc.Bacc(None, target_bir_lowering=False, debug=False,
                   num_devices=N_CORES)

    KC = D // 128                # contraction chunks for the projections
    NSLAB = S // SLAB
    Exp = mybir.ActivationFunctionType.Exp

    # ---- kernel I/O ----
    xT4 = nc.declare_dram_parameter("xT4", [NQ, D, QT], BF16, isOutput=False)
    wqT = nc.declare_dram_parameter("wqT", [D, DL], BF16, isOutput=False)
    wkT = nc.declare_dram_parameter("wkT", [D, DL], BF16, isOutput=False)
    wvT = nc.declare_dram_parameter("wvT", [D, DL], BF16, isOutput=False)
    woT = nc.declare_dram_parameter("woT", [DL, D], BF16, isOutput=False)
    cos2 = nc.declare_dram_parameter("cos2", [128, S], BF16, isOutput=False)
    sins = nc.declare_dram_parameter("sins", [128, S], BF16, isOutput=False)
    idm = nc.declare_dram_parameter("idm", [128, 2 * 128], BF16,
                                    isOutput=False)
    tri4 = nc.declare_dram_parameter("tri4", [128, 2 * 128], BF16,
                                     isOutput=False)
    sel = nc.declare_dram_parameter("sel", [128, 128], BF16, isOutput=False)
    out4 = nc.declare_dram_parameter("out4", [NQ, D, QT], BF16, isOutput=True)

    with tile.TileContext(nc) as tc:
        with tc.tile_pool(name="persist", bufs=1) as pp:
            xt_sb = [pp.tile([128, S], BF16, tag=f"xt{k}", name=f"xt{k}")
                     for k in range(KC)]
            qt_sb = pp.tile([128, 2, S], BF16, tag="qt")
            kt_sb = pp.tile([128, 2, S], BF16, tag="kt")
            v_sb = pp.tile([128, S // 128, 65 * HEADS_PER_CORE], BF16, tag="v")
            ctx_sb = pp.tile([128, 2, S], BF16, tag="ctx")
            cos_sb = pp.tile([128, S], BF16, tag="cos")
            sin_sb = pp.tile([128, S], BF16, tag="sin")
            id_sb = pp.tile([128, 2, 128], BF16, tag="idm")
            tri_sb = pp.tile([128, 2, 128], BF16, tag="tri4")
            sel_sb = pp.tile([128, 128], BF16, tag="sel")
            wq_sb = pp.tile([128, KC, DL], BF16, tag="wq")
            wk_sb = pp.tile([128, KC, DL], BF16, tag="wk")
            wv_sb = pp.tile([128, KC, DL], BF16, tag="wv")
            wo_sb = pp.tile([128, DL // 128, D], BF16, tag="wo")
            # ping-pong rowsum tiles (memset once so the unused partition
            # rows always hold 1.0 -> reciprocal stays finite)
            rs_pp = [[pp.tile([128, QT], F32, tag=f"rs{i}{j}",
                               name=f"rs{i}{j}") for j in range(2)]
                     for i in range(2)]
            rsr_pp = [[pp.tile([128, QT], F32, tag=f"rsr{i}{j}",
                               name=f"rsr{i}{j}") for j in range(2)]
                      for i in range(2)]
            rsrb_pp = [[pp.tile([128, QT], BF16, tag=f"rsb{i}{j}",
                                name=f"rsb{i}{j}") for j in range(2)]
                       for i in range(2)]
            # dedicated exp buffers for diagonal blocks: [head][rel r]
            # keeps cols < 128*r permanently 0 (exp never writes them)
            es_diag = [[pp.tile([128, QT], BF16, tag=f"esd{h}_{r}",
                                name=f"esd{h}_{r}")
                        for r in range(QT // KT)] for h in range(2)]

            # ---- loads, ordered so the PE can start at minimum latency:
            # wv + first x columns feed v_proj(0) almost immediately ----
            def dma_x(quarter, col_lo, col_hi):
                src = xT4.ap()[quarter].rearrange("(c p) s -> c p s", p=128)
                for k in range(KC):
                    nc.sync.dma_start(
                        xt_sb[k][:, QT * quarter + col_lo:
                                 QT * quarter + col_hi],
                        src[k][:, col_lo:col_hi])

            nc.sync.dma_start(
                wv_sb[:], wvT.ap().rearrange("(c p) m -> p c m", p=128))
            dma_x(0, 0, 128)
            dma_x(0, 128, QT)
            for sb, dram in ((wk_sb, wkT), (wq_sb, wqT)):
                nc.sync.dma_start(
                    sb[:], dram.ap().rearrange("(c p) m -> p c m", p=128))
            dma_x(1, 0, QT)
            nc.sync.dma_start(cos_sb[:], cos2.ap())
            nc.sync.dma_start(sin_sb[:], sins.ap())
            dma_x(2, 0, QT)
            dma_x(3, 0, QT)
            nc.sync.dma_start(
                wo_sb[:], woT.ap().rearrange("(c p) m -> p c m", p=128))
            nc.sync.dma_start(
                id_sb[:], idm.ap().rearrange("p (c m) -> p c m", c=2))
            nc.sync.dma_start(
                tri_sb[:], tri4.ap().rearrange("p (c m) -> p c m", c=2))
            nc.sync.dma_start(sel_sb[:], sel.ap())

            # only the ones-columns of V need the memset; head data is
            # fully overwritten by the projection copies
            for h in range(HEADS_PER_CORE):
                nc.gpsimd.memset(v_sb[:, :, 65 * h + 64:65 * h + 65], 1.0)
            for i in range(2):
                for j in range(2):
                    nc.gpsimd.memset(rs_pp[i][j][:], 1.0)
            for h in range(2):
                for r in range(1, QT // KT):
                    nc.gpsimd.memset(es_diag[h][r][:, 0:KT * r], 0.0)

            # ================= Phase 1: projections =================
            with (
                tc.tile_pool(name="p1ps", bufs=3, space="PSUM") as p1ps,
                tc.tile_pool(name="p1sb", bufs=3) as p1sb,
            ):
                def v_proj(si):
                    ps = p1ps.tile([128, DL], F32, tag="v", name=f"vps{si}")
                    for k in range(KC):
                        nc.tensor.matmul(
                            ps[:],
                            xt_sb[k][:, 128 * si:128 * (si + 1)],
                            wv_sb[:, k, :],
                            start=(k == 0), stop=(k == KC - 1),
                        )
                    # scalar: vector is saturated by the RoPE chain in
                    # phase 1 and would gate the v PSUM ring
                    nc.scalar.copy(
                        v_sb[:, si].rearrange(
                            "p (h c) -> p h c", c=65)[:, :, 0:64],
                        ps.rearrange("p (h c) -> p h c", c=64),
                    )

                for si in range(4):      # early PE work while x still loads
                    v_proj(si)
                for half in range(NSLAB):
                    ssl = slice(SLAB * half, SLAB * (half + 1))
                    # K^T then Q^T with fused RoPE, on [128, SLAB] slabs
                    for dst, wsb in ((kt_sb, wk_sb), (qt_sb, wq_sb)):
                        for m in range(2):
                            rin = p1sb.tile([128, SLAB], BF16, tag="rin")
                            for qs in range(SLAB // QT):
                                ps = p1ps.tile([128, QT], F32, tag="qk")
                                for k in range(KC):
                                    nc.tensor.matmul(
                                        ps[:],
                                        wsb[:, k, 128 * m:128 * (m + 1)],
                                        xt_sb[k][:, SLAB * half + QT * qs:
                                                 SLAB * half + QT * (qs + 1)],
                                        start=(k == 0), stop=(k == KC - 1),
                                    )
                                nc.scalar.copy(
                                    rin[:, QT * qs:QT * (qs + 1)], ps[:])
                            tmp = p1sb.tile([128, SLAB], BF16, tag="rtmp")
                            for q in range(4):   # partner * sign(sin)
                                src = (q + 1 if q % 2 == 0 else q - 1) * 32
                                nc.vector.tensor_mul(
                                    tmp[32 * q:32 * (q + 1), :],
                                    rin[src:src + 32, :],
                                    sin_sb[src:src + 32, ssl],
                                )
                            qc = p1sb.tile([128, SLAB], BF16, tag="rqc")
                            nc.vector.tensor_mul(qc[:], rin[:], cos_sb[:, ssl])
                            nc.vector.tensor_add(dst[:, m, ssl], qc[:], tmp[:])

                    # V (natural layout, interleaved with the ones columns)
                    for si in range(SLAB // 128 * half,
                                    SLAB // 128 * (half + 1)):
                        if si >= 4:
                            v_proj(si)

            # ========== Phase 2+3+4: attention / normalize / project ========
            with (
                # 4 score banks decouple the PE from the exp drain by a
                # full k-step; the AV ring runs at 2 because the ot
                # evacuation frees it right after each section
                tc.tile_pool(name="scps", bufs=4, space="PSUM") as scps,
                tc.tile_pool(name="ops", bufs=2, space="PSUM") as ops,
                tc.tile_pool(name="obps", bufs=2, space="PSUM") as obps,
                tc.tile_pool(name="essb", bufs=8) as essb,
                tc.tile_pool(name="p4sb", bufs=3) as p4sb,
            ):
                # deferred emissions so the PE never waits on the
                # reciprocal chain: flushed 2 k-steps into the NEXT section
                pending = []

                def emit_outproj(qi):
                    qsl = slice(QT * qi, QT * (qi + 1))
                    for e in range(D // 128):
                        ps = obps.tile([128, QT], F32, tag="op",
                                       name=f"op{qi}_{e}")
                        for kc in range(DL // 128):
                            nc.tensor.matmul(
                                ps[:],
                                wo_sb[:, kc, 128 * e:128 * (e + 1)],
                                ctx_sb[:, kc, qsl],
                                start=(kc == 0), stop=(kc == DL // 128 - 1),
                            )
                        # vector only: anything on the scalar queue that
                        # waits on a PE matmul head-of-line-blocks the exp
                        # stream behind it
                        yt = p4sb.tile([128, QT], BF16, tag="yt")
                        nc.vector.tensor_copy(yt[:], ps[:])
                        nc.sync.dma_start(
                            out4.ap()[qi, 128 * e:128 * (e + 1), :], yt[:])

                for qi in range(NQ):
                    qsl = slice(QT * qi, QT * (qi + 1))
                    diag0 = (QT * qi) // KT      # first diagonal k-block
                    live = min(NKB, diag0 + QT // KT)
                    for j in range(2):           # head pairs
                        o_ps = [ops.tile([65, QT], F32, tag="o",
                                         name=f"o{qi}{j}{h}")
                                for h in range(2)]
                        # process diagonal blocks first: the AV group then
                        # starts (r=0) and stops (last off-diag) with
                        # full-width instructions, so the sliced middle
                        # blocks keep the accumulation group sound
                        ks = list(range(diag0, live)) + list(range(diag0))
                        nsteps = len(ks)

                        def emit_av(step_, kb_, h_, es_):
                            hl = 2 * j + h_
                            # slice diagonal AV only when a full-width
                            # off-diagonal block closes the group (qi>0)
                            ca = (KT * (kb_ - diag0)
                                  if (kb_ > diag0 and diag0 > 0) else 0)
                            nc.tensor.matmul(
                                o_ps[h_][:, ca:],
                                v_sb[:, kb_, 65 * hl:65 * hl + 65],
                                es_[:, ca:],
                                start=(step_ == 0), stop=(step_ == nsteps - 1),
                                skip_group_check=(ca > 0),
                            )

                        prev = None
                        for idx, kb in enumerate(ks):
                            diag = kb >= diag0
                            r = kb - diag0 if diag else 0
                            c0 = KT * r
                            cur = []
                            for h01 in range(2):
                                p0 = 64 * h01
                                kt_ap = kt_sb[p0:p0 + 64, j,
                                              128 * kb:128 * (kb + 1)]
                                sc = scps.tile([128, QT], F32, tag="sc")
                                if diag:
                                    # in-block triangle: scores then an
                                    # additive -240 * tri accumulate, as a
                                    # closed group on [c0, c0+KT); the
                                    # remaining live columns are their own
                                    # single-instruction group
                                    nc.tensor.matmul(
                                        sc[:, c0:c0 + KT], kt_ap,
                                        qt_sb[p0:p0 + 64, j,
                                              QT * qi + c0:
                                              QT * qi + c0 + KT],
                                        start=True, stop=False,
                                    )
                                    for part in range(2):
                                        nc.tensor.matmul(
                                            sc[:, c0:c0 + KT],
                                            id_sb[p0:p0 + 64, part, :],
                                            tri_sb[p0:p0 + 64, part, :],
                                            start=False, stop=(part == 1),
                                        )
                                    if c0 + KT < QT:
                                        nc.tensor.matmul(
                                            sc[:, c0 + KT:], kt_ap,
                                            qt_sb[p0:p0 + 64, j,
                                                  QT * qi + c0 + KT:
                                                  QT * (qi + 1)],
                                            start=True, stop=True,
                                        )
                                    es = es_diag[h01][r]
                                else:
                                    nc.tensor.matmul(
                                        sc[:], kt_ap,
                                        qt_sb[p0:p0 + 64, j, qsl],
                                        start=True, stop=True,
                                    )
                                    es = essb.tile([128, QT], BF16, tag="es")
                                nc.scalar.activation(
                                    es[:, c0:], sc[:, c0:], Exp, scale=SCALE)
                                cur.append((idx, kb, h01, es))
                            if idx == 2 and pending:
                                for fn in pending:
                                    fn()
                                pending.clear()
                            if prev is not None:
                                for args in prev:
                                    emit_av(*args)
                            prev = cur
                        for args in prev:
                            emit_av(*args)
                        # rowsums -> reciprocal now; broadcast + normalize
                        # deferred into the next section's k-loop
                        rs = rs_pp[qi % 2][j]
                        rsr = rsr_pp[qi % 2][j]
                        rsrb = rsrb_pp[qi % 2][j]
                        nc.vector.tensor_copy(rs[0:1, :], o_ps[0][64:65, :])
                        nc.vector.tensor_copy(rs[32:33, :], o_ps[1][64:65, :])
                        # evacuate the AV context to SBUF first so the o_ps
                        # ring frees before the reciprocal latency (the mul
                        # below may read only one PSUM operand anyway)
                        ot = p4sb.tile([128, QT], F32, tag="ot")
                        nc.vector.tensor_copy(ot[0:64, :], o_ps[0][0:64, :])
                        nc.vector.tensor_copy(ot[64:128, :], o_ps[1][0:64, :])
                        nc.vector.reciprocal_approx_fast(rsr[:], rs[:])
                        nc.vector.tensor_copy(rsrb[0:33, :], rsr[0:33, :])

                        def mk_norm(qi=qi, j=j, ot=ot, rsrb=rsrb, qsl=qsl):
                            def emit():
                                # broadcast the two recip rows (partitions
                                # 0 / 32) to partitions 0-63 / 64-127 via
                                # the selector matmul
                                bps = obps.tile([128, QT], F32, tag="op",
                                                name=f"bps{qi}{j}")
                                nc.tensor.matmul(
                                    bps[:],
                                    sel_sb[0:33, :],
                                    rsrb[0:33, :],
                                    start=True, stop=True,
                                )
                                nc.vector.tensor_mul(
                                    ctx_sb[:, j, qsl], ot[:], bps[:])
                            return emit

                        pending.append(mk_norm())
                        if j == 0 and qi > 0:
                            # outproj provides PE cover while the recip
                            # chain completes; flush the deferred
                            # normalize right after it
                            emit_outproj(qi - 1)
                            for fn in pending:
                                fn()
                            pending.clear()
                for fn in pending:       # final head pair's normalize
                    fn()
                pending.clear()
                emit_outproj(NQ - 1)

    nc.compile()
    return nc


def host_inputs(x, mask, w_qkv, w_out):
    """Shard + pre-transpose inputs per core. Returns in_maps list."""
    del mask  # causality is baked into the kernel (reference mask is tril)
    inv = 1.0 / (10000.0 ** (np.arange(0, HD, 2, dtype=np.float64) / HD))
    t = np.arange(S, dtype=np.float64)
    fr = np.outer(t, inv)
    emb = np.concatenate([fr, fr], axis=1)          # [S, hd]
    cosT = np.cos(emb).T.astype(np.float32)         # [hd, S]
    sinT = np.sin(emb).T.astype(np.float32)
    cos2 = np.vstack([cosT, cosT]).astype(NP_BF16)
    # value at partition p = sin factor applied to SOURCE partition p
    sins = np.vstack([sinT[32:], -sinT[:32],
                      sinT[32:], -sinT[:32]]).astype(NP_BF16)
    kk = np.arange(128)
    tri = (kk[None, :] < kk[:, None]).astype(np.float32)  # [k, q] 1 if q < k
    idm_np = np.zeros((128, 2, 128), dtype=np.float32)
    tri4_np = np.zeros((128, 2, 128), dtype=np.float32)
    for p in range(128):
        for c in range(2):
            idm_np[p, c, 64 * c + (p % 64)] = MASK_RAW
            tri4_np[p, c, :] = tri[(p % 64) + 64 * c, :]
    idm_np = idm_np.reshape(128, 256).astype(NP_BF16)
    tri4_np = tri4_np.reshape(128, 256).astype(NP_BF16)
    sel = np.zeros((128, 128), dtype=NP_BF16)            # recip broadcast
    sel[0, 0:64] = 1.0
    sel[32, 64:128] = 1.0

    in_maps = []
    for c in range(N_CORES):
        b, g = divmod(c, GROUPS_PER_BATCH)
        rows = slice(DL * g, DL * (g + 1))
        xb = np.ascontiguousarray(x[b].T).astype(NP_BF16)       # [D, S]
        xT4 = np.ascontiguousarray(
            xb.reshape(D, NQ, QT).transpose(1, 0, 2))           # [NQ, D, QT]
        in_maps.append({
            "xT4": xT4,
            "wqT": np.ascontiguousarray(w_qkv[rows, :].T).astype(NP_BF16),
            "wkT": np.ascontiguousarray(w_qkv[D:][rows, :].T).astype(NP_BF16),
            "wvT": np.ascontiguousarray(w_qkv[2 * D:][rows, :].T).astype(NP_BF16),
            "woT": np.ascontiguousarray(w_out[:, rows].T).astype(NP_BF16),
            "cos2": cos2,
            "sins": sins,
            "idm": idm_np,
            "tri4": tri4_np,
            "sel": sel,
        })
    return in_maps


_NC_CACHE = {}


def _get_nc():
    if "nc" not in _NC_CACHE:
        _NC_CACHE["nc"] = build_nc()
    return _NC_CACHE["nc"]


def _np_reference(x, mask, w_qkv, w_out):
    """Plain numpy fallback (used only if mask is not causal-tril)."""
    q = x @ w_qkv[:D].T
    k = x @ w_qkv[D:2 * D].T
    v = x @ w_qkv[2 * D:].T

    def split(t):
        return t.reshape(B, S, H, HD).transpose(0, 2, 1, 3)

    q, k, v = split(q), split(k), split(v)
    inv = 1.0 / (10000.0 ** (np.arange(0, HD, 2, dtype=np.float64) / HD))
    fr = np.outer(np.arange(S, dtype=np.float64), inv)
    emb = np.concatenate([fr, fr], axis=1)
    cos = np.cos(emb).astype(np.float32)[None, None]
    sin = np.sin(emb).astype(np.float32)[None, None]

    def rot(t):
        return np.concatenate([-t[..., HD // 2:], t[..., :HD // 2]], axis=-1)

    q = q * cos + rot(q) * sin
    k = k * cos + rot(k) * sin
    a = np.einsum("bhqd,bhkd->bhqk", q, k) * SCALE
    a = np.where(mask, a, -np.inf)
    a = a - a.max(axis=-1, keepdims=True)
    a = np.exp(a)
    a /= a.sum(axis=-1, keepdims=True)
    o = np.einsum("bhqk,bhkd->bhqd", a, v)
    o = o.transpose(0, 2, 1, 3).reshape(B, S, D)
    return (o @ w_out.T).astype(np.float32)


def kernel(x, mask, w_qkv, w_out):
    x = np.asarray(x)
    w_qkv = np.asarray(w_qkv)
    w_out = np.asarray(w_out)
    if mask is not None:
        m = np.asarray(mask).reshape(S, S)
        if not np.array_equal(m, np.tril(np.ones((S, S), dtype=bool))):
            return _np_reference(x, m.reshape(1, 1, S, S), w_qkv, w_out)
    nc = _get_nc()
    in_maps = host_inputs(x, mask, w_qkv, w_out)
    res = run_bass_kernel_spmd(nc, in_maps, core_ids=list(range(N_CORES)))
    # each result: out4 [NQ, D, QT] partial (head-group share of y[b].T)
    outs = [r["out4"].astype(np.float32) for r in res.results]
    y = np.empty((B, S, D), dtype=np.float32)
    for b in range(B):
        acc = sum(outs[GROUPS_PER_BATCH * b + g]
                  for g in range(GROUPS_PER_BATCH))      # [NQ, D, QT]
        y[b] = acc.transpose(1, 0, 2).reshape(D, S).T
    return y

